# revision 1
# baseline (speedup 1.0000x reference)
"""Binarized 3x3 conv (GeneralConv2d) on 8 NeuronCores.

y[b,o,h,w] = mean_abs(w[o]) * sum_{c,kh,kw} sign(w[o,c,kh,kw]) * x[b,c,h+kh-1,w+kw-1]

Data-parallel over batch: 4 images per core on 8 cores; the tiny binarized
weight is replicated.  Per core the conv is a sum of 18 shifted 128x128
GEMMs per output chunk, accumulated in PSUM in bf16 (exact +-1 weights,
bf16-rounded x), scaled by the per-channel mean-abs on eviction.
"""

import numpy as np

from contextlib import ExitStack

import concourse.bass as bass
import concourse.mybir as mybir
from concourse import bacc
import concourse.tile as tile
from concourse.masks import make_identity

dt = mybir.dt
OUT_C = 256
IN_C = 256
KH = KW = 3
KK = KH * KW           # 9
CKK = IN_C * KK        # 2304
CHK = P128 = 128
P = 128
CC = IN_C // P         # 2 in-channel chunks
OO = OUT_C // P        # 2 out-channel chunks
QC = CKK // CC         # 1152 columns per (oo,cc) quarter


def _build_conv_nc(imgs: int, H: int, W: int, hchunk: int, psum_bufs: int = 7,
                  ostage_bufs: int = 4, gsz: int = 4, tp_bufs: int = 1):
    assert H % hchunk == 0
    nch = H // hchunk
    Hp, Wp = H + 2, W + 2
    nc = bacc.Bacc("TRN2", target_bir_lowering=False, debug=False,
                   enable_asserts=False, num_devices=8)
    x = nc.declare_dram_parameter("x", [imgs, IN_C, H, W], dt.float32, isOutput=False)
    w = nc.declare_dram_parameter("w", [OUT_C * CKK, 1], dt.float32, isOutput=False)
    y = nc.declare_dram_parameter("y", [imgs, OUT_C, H, W], dt.float32, isOutput=True)

    w2d = w.rearrange("(o r) one -> o (r one)", r=CKK)   # [256, 2304]

    with tile.TileContext(nc) as tc, ExitStack() as ctx:
        consts = ctx.enter_context(tc.tile_pool(name="consts", bufs=1))
        ident = consts.tile([P, P], dt.bfloat16)
        make_identity(nc, ident)
        zrow = consts.tile([P, 2 * Wp], dt.bfloat16)
        nc.vector.memset(zrow, 0.0)

        wprep = ctx.enter_context(tc.tile_pool(name="wprep", bufs=1))
        w_sb = wprep.tile([P, OO, CKK], dt.float32)
        sgn_sb = wprep.tile([P, OO, CKK], dt.bfloat16)
        scale_sb = wprep.tile([P, OO], dt.float32)
        sgn_v = sgn_sb.rearrange("p oo (c k) -> p oo c k", k=KK)

        tpool = ctx.enter_context(tc.tile_pool(name="tpsum", bufs=tp_bufs, space="PSUM"))
        wtp = ctx.enter_context(tc.tile_pool(name="wtiles", bufs=OO * CC * KK))
        xp = ctx.enter_context(tc.tile_pool(name="xtiles", bufs=imgs * CC))

        xt = {}

        def load_x(img):
            for cc in range(CC):
                t = xp.tile([P, Hp, Wp], dt.bfloat16)
                # Halo zeros on the (idle-at-startup) scalar engine, keeping
                # DVE free for the sign ops the transposes wait on.  The
                # interior halo columns (w=57 of row h, w=0 of row h+1) are
                # adjacent in the flat layout, so three contiguous strips
                # cover the whole halo.
                tf = t.rearrange("p h w -> p (h w)")
                nc.scalar.copy(tf[:, 0:Wp], zrow[:, 0:Wp])
                nc.scalar.copy(tf[:, (Hp - 1) * Wp:Hp * Wp], zrow[:, 0:Wp])
                mid = tf[:, Wp - 1:Wp - 1 + (Hp - 1) * Wp].rearrange(
                    "p (h w) -> p h w", w=Wp)[:, :, 0:2]
                nc.scalar.copy(mid, zrow[:, 0:2 * (Hp - 1)].rearrange(
                    "p (h w) -> p h w", w=2))
                # Two half-row DMAs land on different queues -> parallel
                # transfer, halving time-to-ready for the first conv matmul.
                h2 = H // 2
                nc.gpsimd.dma_start(out=t[:, 1:h2 + 1, 1:W + 1],
                                    in_=x[img, cc * P:(cc + 1) * P, 0:h2])
                nc.gpsimd.dma_start(out=t[:, h2 + 1:H + 1, 1:W + 1],
                                    in_=x[img, cc * P:(cc + 1) * P, h2:H])
                xt[(img, cc)] = t

        wt = {}

        def prep_w_quarter(oo, cc):
            # DMA the (oo, cc) quarter of w: rows o=oo*128+p, cols cc*1152..+1152
            q2 = QC // 2
            for h in range(2):
                nc.sync.dma_start(
                    out=w_sb[:, oo, cc * QC + h * q2:cc * QC + (h + 1) * q2],
                    in_=w2d[oo * P:(oo + 1) * P,
                            cc * QC + h * q2:cc * QC + (h + 1) * q2])
            nc.vector.tensor_scalar(
                out=sgn_sb[:, oo, cc * QC:(cc + 1) * QC],
                in0=w_sb[:, oo, cc * QC:(cc + 1) * QC],
                scalar1=0.0, scalar2=2.0,
                op0=mybir.AluOpType.is_ge, op1=mybir.AluOpType.mult)
            nc.vector.tensor_scalar_add(
                sgn_sb[:, oo, cc * QC:(cc + 1) * QC],
                sgn_sb[:, oo, cc * QC:(cc + 1) * QC], -1.0)
            for k in range(KK):
                tp = tpool.tile([P, P], dt.bfloat16)
                nc.tensor.transpose(tp, sgn_v[:, oo, cc * P:(cc + 1) * P, k], ident)
                t = wtp.tile([P, P], dt.bfloat16)
                nc.vector.tensor_copy(out=t, in_=tp)
                wt[(oo, cc, k)] = t

        def reduce_scale(oo):
            # Per-out-channel scale column (runs on DVE behind the conv).
            nc.vector.tensor_reduce(
                out=scale_sb[:, oo:oo + 1], in_=w_sb[:, oo, :],
                axis=mybir.AxisListType.X,
                op=mybir.AluOpType.add, apply_absolute_value=True)
            nc.vector.tensor_scalar_mul(
                scale_sb[:, oo:oo + 1], scale_sb[:, oo:oo + 1], 1.0 / CKK)

        pp = ctx.enter_context(tc.tile_pool(name="psum", bufs=psum_bufs, space="PSUM"))
        op = ctx.enter_context(tc.tile_pool(name="ostage", bufs=ostage_bufs))

        def mm(ps, img, oo, cc, ih, k, n):
            ki, kj = divmod(k, KW)
            rhs = xt[(img, cc)][
                :, ih * hchunk + ki: ih * hchunk + ki + hchunk, kj: kj + W]
            nc.tensor.matmul(ps, lhsT=wt[(oo, cc, k)], rhs=rhs,
                             start=(n == 0), stop=(n == CC * KK - 1))

        def conv_a(img, oo, tiles):
            # Pass A: all cc0 taps for the group's tiles (start accumulation).
            group = {}
            for ih in tiles:
                ps = pp.tile([P, hchunk * W], dt.float32,
                             name=f"ps_{img}_{oo}_{ih}", tag="ps")
                group[ih] = ps
                for k in range(KK):
                    mm(ps, img, oo, 0, ih, k, n=k)
            return group

        def conv_b(img, oo, group):
            # Pass B: cc1 taps, then scale + store.
            for ih, ps in group.items():
                for k in range(KK):
                    mm(ps, img, oo, 1, ih, k, n=KK + k)
                st = op.tile([P, hchunk * W], dt.float32,
                             name=f"st_{img}_{oo}_{ih}", tag="st")
                nc.scalar.mul(st, ps, scale_sb[:, oo:oo + 1])
                nc.sync.dma_start(
                    out=y[img, oo * P:(oo + 1) * P,
                          ih * hchunk:(ih + 1) * hchunk, :],
                    in_=st)

        def conv(img, oo, skip=0):
            for g0 in range(skip, nch, gsz):
                tiles = list(range(g0, min(g0 + gsz, nch)))
                conv_b(img, oo, conv_a(img, oo, tiles))

        # Emission order doubles as per-engine program order (PE is in-order):
        # transpose batches alternate with conv half-passes so each batch's
        # DVE-side prep (sign + copies) completes during the previous conv
        # burst and no transpose wait stalls ready conv matmuls behind it.
        # Groups of `gsz` < psum_bufs keep consecutive groups on disjoint
        # PSUM banks, so pass A never WAR-waits on the previous group's
        # evictions.
        load_x(0)
        prep_w_quarter(0, 0)
        a1 = conv_a(0, 0, list(range(min(gsz, nch))))
        prep_w_quarter(0, 1)
        reduce_scale(0)
        if imgs > 1:
            load_x(1)
        conv_b(0, 0, a1)
        prep_w_quarter(1, 0)
        if nch > gsz:
            a2 = conv_a(0, 0, list(range(gsz, min(2 * gsz, nch))))
            prep_w_quarter(1, 1)
            reduce_scale(1)
            conv_b(0, 0, a2)
            conv(0, 0, skip=2 * gsz)
        else:
            prep_w_quarter(1, 1)
            reduce_scale(1)
        for img in range(2, imgs):
            load_x(img)
        conv(0, 1)
        for img in range(1, imgs):
            conv(img, 0)
            conv(img, 1)
    nc.compile()
    return nc


BATCH, H, W = 32, 56, 56
N_CORES = 8
IMGS = BATCH // N_CORES
_NC_CACHE = {}


def _get_nc():
    key = (IMGS, H, W)
    if key not in _NC_CACHE:
        _NC_CACHE[key] = _build_conv_nc(IMGS, H, W, hchunk=8, psum_bufs=7,
                                        gsz=4, tp_bufs=1)
    return _NC_CACHE[key]


def kernel(**inputs) -> np.ndarray:
    from concourse.bass_utils import run_bass_kernel_spmd

    x = np.ascontiguousarray(np.asarray(inputs["x"], dtype=np.float32))
    weight = np.ascontiguousarray(np.asarray(inputs["weight"], dtype=np.float32))
    assert x.shape == (BATCH, IN_C, H, W), x.shape
    assert weight.shape == (OUT_C * CKK, 1), weight.shape

    nc = _get_nc()
    in_maps = [
        {"x": x[c * IMGS:(c + 1) * IMGS], "w": weight}
        for c in range(N_CORES)
    ]
    res = run_bass_kernel_spmd(nc, in_maps, core_ids=list(range(N_CORES)))
    return np.concatenate([res.results[c]["y"] for c in range(N_CORES)], axis=0)



# revision 3
# speedup vs baseline: 2.3175x; 2.3175x over previous
"""Binarized 3x3 conv (GeneralConv2d) on 8 NeuronCores.

y[b,o,h,w] = mean_abs(w[o]) * sum_{c,kh,kw} sign(w[o,c,kh,kw]) * x[b,c,h+kh-1,w+kw-1]

The module initializes w = rand()*0.001, so every weight is non-negative and
sign(w) == +1 identically.  The conv then collapses to a rank-1 form

    y[b,o,:,:] = scale_o * S[b,:,:],   S = 3x3 box filter of sum_c x[b,c]

which is DMA-bound rather than compute-bound.  kernel() verifies the
all-non-negative precondition on the host and falls back to the general
binarized-conv kernel if it ever fails.

Fast path, data-parallel over batch (4 images per core):
 - x is DMA'd f32 with full-width contiguous descriptors into a flat staging
   tile, then converted to bf16 into a zero-padded [128, 58, 58] tile (the
   conversion pass does the padding relayout for free; DMAing the padded
   layout directly would halve DMA bandwidth on 224B descriptors).
 - PE: per 8-row chunk, 6 accumulating bf16 matmuls (2 channel chunks x 3
   vertical taps) with an all-ones stationary operand compute the channel +
   vertical sum, replicated across all 128 partitions, into PSUM [128, 58*8].
 - DVE: copy PSUM->SBUF, then the horizontal 3-tap as two strided adds
   (the zero pad columns make row edges exact).
 - Act: per-partition multiply by scale[oo*128+p] yields the output channel
   chunk directly (the sum is partition-replicated); SP DMAs it out.
"""

import numpy as np

from contextlib import ExitStack

import concourse.bass as bass
import concourse.mybir as mybir
from concourse import bacc
import concourse.tile as tile
from concourse.masks import make_identity

dt = mybir.dt
OUT_C = 256
IN_C = 256
KH = KW = 3
KK = KH * KW           # 9
CKK = IN_C * KK        # 2304
P = 128
CC = IN_C // P         # 2 in-channel chunks
OO = OUT_C // P        # 2 out-channel chunks
QC = CKK // CC         # 1152 columns per (oo,cc) quarter

BATCH, H, W = 32, 56, 56
N_CORES = 8
IMGS = BATCH // N_CORES

HC = 8                 # output rows per PSUM chunk
NCH = H // HC          # 7 chunks per image
Hp = H + 2
Wp = W + 2
FCH = HC * Wp          # 464 PSUM columns per chunk (58-wide rows)


def _build_rank1_nc(imgs: int):
    nc = bacc.Bacc("TRN2", target_bir_lowering=False, debug=False,
                   enable_asserts=False, num_devices=8)
    x = nc.declare_dram_parameter("x", [imgs, IN_C, H, W], dt.float32,
                                  isOutput=False)
    w = nc.declare_dram_parameter("w", [OUT_C * CKK, 1], dt.float32,
                                  isOutput=False)
    y = nc.declare_dram_parameter("y", [imgs, OUT_C, H, W], dt.float32,
                                  isOutput=True)

    x2d = x.rearrange("i c h w -> i c (h w)")
    y2d = y.rearrange("i c h w -> i c (h w)")
    w2d = w.rearrange("(o r) one -> o (r one)", r=CKK)   # [256, 2304]

    with tile.TileContext(nc) as tc, ExitStack() as ctx:
        consts = ctx.enter_context(tc.tile_pool(name="consts", bufs=1))
        ones = consts.tile([P, P], dt.bfloat16)
        nc.vector.memset(ones, 1.0)
        zrow = consts.tile([P, 2 * Wp], dt.bfloat16)
        nc.vector.memset(zrow, 0.0)

        wq = ctx.enter_context(tc.tile_pool(name="wq", bufs=1))
        w_sb = wq.tile([P, OO, CKK], dt.float32)
        scale_sb = wq.tile([P, OO], dt.float32)

        def prep_scale():
            for oo in range(OO):
                nc.sync.dma_start(out=w_sb[:, oo, :],
                                  in_=w2d[oo * P:(oo + 1) * P, :])
            for oo in range(OO):
                nc.vector.tensor_reduce(
                    out=scale_sb[:, oo:oo + 1], in_=w_sb[:, oo, :],
                    axis=mybir.AxisListType.X,
                    op=mybir.AluOpType.add, apply_absolute_value=True)
            nc.vector.tensor_scalar_mul(scale_sb, scale_sb, 1.0 / CKK)

        # f32 staging tiles (full-bandwidth contiguous DMA target).
        sxp = ctx.enter_context(tc.tile_pool(name="xstage", bufs=4))
        # bf16 padded tiles; all 8 stay resident so there are no reuse stalls.
        xp = ctx.enter_context(tc.tile_pool(name="xpad", bufs=imgs * CC))
        xt = {}

        def load_x(img):
            stg = {}
            h2 = H // 2
            # Issue all four half-image DMAs first (independent), then the
            # conversions in arrival order, so Pool never idles on a wait.
            for cc in range(CC):
                sx = sxp.tile([P, H * W], dt.float32)
                t = xp.tile([P, Hp, Wp], dt.bfloat16)
                tf = t.rearrange("p h w -> p (h w)")
                # Halo zeroing on Act (idle early): top row, bottom row, and
                # the adjacent (r,57)/(r+1,0) interior column pairs.
                nc.scalar.copy(tf[:, 0:Wp], zrow[:, 0:Wp])
                nc.scalar.copy(tf[:, (Hp - 1) * Wp:Hp * Wp], zrow[:, 0:Wp])
                mid = tf[:, Wp - 1:Wp - 1 + (Hp - 1) * Wp].rearrange(
                    "p (h w) -> p h w", w=Wp)[:, :, 0:2]
                nc.scalar.copy(mid, zrow[:, 0:2 * (Hp - 1)].rearrange(
                    "p (h w) -> p h w", w=2))
                stg[cc] = (sx, t)
                xt[(img, cc)] = t
            for hh in range(2):
                for cc in range(CC):
                    sx, _ = stg[cc]
                    nc.gpsimd.dma_start(
                        out=sx[:, hh * h2 * W:(hh + 1) * h2 * W],
                        in_=x2d[img, cc * P:(cc + 1) * P,
                                hh * h2 * W:(hh + 1) * h2 * W])
            for hh in range(2):
                for cc in range(CC):
                    sx, t = stg[cc]
                    nc.gpsimd.tensor_copy(
                        out=t[:, 1 + hh * h2:1 + (hh + 1) * h2, 1:W + 1],
                        in_=sx[:, hh * h2 * W:(hh + 1) * h2 * W].rearrange(
                            "p (h w) -> p h w", w=W))

        pp = ctx.enter_context(tc.tile_pool(name="psum", bufs=8, space="PSUM"))
        up = ctx.enter_context(tc.tile_pool(name="utile", bufs=4))
        vp = ctx.enter_context(tc.tile_pool(name="vtile", bufs=4))
        op = ctx.enter_context(tc.tile_pool(name="stile", bufs=6))

        def conv_chunk(img, ih):
            h0 = ih * HC
            ps = pp.tile([P, FCH], dt.float32, name=f"ps_{img}_{ih}", tag="ps")
            n = 0
            for cc in range(CC):
                t = xt[(img, cc)].rearrange("p h w -> p (h w)")
                for kh in range(KH):
                    base = (h0 + kh) * Wp
                    nc.tensor.matmul(ps, lhsT=ones,
                                     rhs=t[:, base:base + FCH],
                                     start=(n == 0), stop=(n == CC * KH - 1))
                    n += 1
            # ps[p, r*58 + wp] = U[h0+r, wp]: channel+vertical sum (replicated
            # across partitions) over the zero-padded width.
            u = up.tile([P, FCH], dt.float32, name=f"u_{img}_{ih}", tag="u")
            nc.vector.tensor_copy(out=u, in_=ps)
            u3 = u.rearrange("p (h w) -> p h w", w=Wp)
            v = vp.tile([P, HC, W], dt.float32, name=f"v_{img}_{ih}", tag="v")
            nc.vector.tensor_add(v, u3[:, :, 0:W], u3[:, :, 1:W + 1])
            nc.vector.tensor_add(v, v, u3[:, :, 2:W + 2])
            vf = v.rearrange("p h w -> p (h w)")
            for oo in range(OO):
                st = op.tile([P, HC * W], dt.float32,
                             name=f"st_{img}_{ih}_{oo}", tag="st")
                nc.scalar.mul(st, vf, scale_sb[:, oo:oo + 1])
                nc.sync.dma_start(
                    out=y2d[img, oo * P:(oo + 1) * P, h0 * W:(h0 + HC) * W],
                    in_=st)

        load_x(0)
        prep_scale()
        load_x(1)
        for ih in range(NCH):
            conv_chunk(0, ih)
        load_x(2)
        for ih in range(NCH):
            conv_chunk(1, ih)
        load_x(3)
        for img in range(2, imgs):
            for ih in range(NCH):
                conv_chunk(img, ih)
    nc.compile()
    return nc


# ---------------------------------------------------------------------------
# General fallback: full binarized conv (sum of 18 shifted GEMMs), used only
# if the weight tensor ever contains a negative value.
# ---------------------------------------------------------------------------


def _build_conv_nc(imgs: int, H: int, W: int, hchunk: int, psum_bufs: int = 7,
                  ostage_bufs: int = 4, gsz: int = 4, tp_bufs: int = 1):
    assert H % hchunk == 0
    nch = H // hchunk
    Hp, Wp = H + 2, W + 2
    nc = bacc.Bacc("TRN2", target_bir_lowering=False, debug=False,
                   enable_asserts=False, num_devices=8)
    x = nc.declare_dram_parameter("x", [imgs, IN_C, H, W], dt.float32, isOutput=False)
    w = nc.declare_dram_parameter("w", [OUT_C * CKK, 1], dt.float32, isOutput=False)
    y = nc.declare_dram_parameter("y", [imgs, OUT_C, H, W], dt.float32, isOutput=True)

    w2d = w.rearrange("(o r) one -> o (r one)", r=CKK)   # [256, 2304]

    with tile.TileContext(nc) as tc, ExitStack() as ctx:
        consts = ctx.enter_context(tc.tile_pool(name="consts", bufs=1))
        ident = consts.tile([P, P], dt.bfloat16)
        make_identity(nc, ident)
        zrow = consts.tile([P, 2 * Wp], dt.bfloat16)
        nc.vector.memset(zrow, 0.0)

        wprep = ctx.enter_context(tc.tile_pool(name="wprep", bufs=1))
        w_sb = wprep.tile([P, OO, CKK], dt.float32)
        sgn_sb = wprep.tile([P, OO, CKK], dt.bfloat16)
        scale_sb = wprep.tile([P, OO], dt.float32)
        sgn_v = sgn_sb.rearrange("p oo (c k) -> p oo c k", k=KK)

        tpool = ctx.enter_context(tc.tile_pool(name="tpsum", bufs=tp_bufs, space="PSUM"))
        wtp = ctx.enter_context(tc.tile_pool(name="wtiles", bufs=OO * CC * KK))
        xp = ctx.enter_context(tc.tile_pool(name="xtiles", bufs=imgs * CC))

        xt = {}

        def load_x(img):
            for cc in range(CC):
                t = xp.tile([P, Hp, Wp], dt.bfloat16)
                tf = t.rearrange("p h w -> p (h w)")
                nc.scalar.copy(tf[:, 0:Wp], zrow[:, 0:Wp])
                nc.scalar.copy(tf[:, (Hp - 1) * Wp:Hp * Wp], zrow[:, 0:Wp])
                mid = tf[:, Wp - 1:Wp - 1 + (Hp - 1) * Wp].rearrange(
                    "p (h w) -> p h w", w=Wp)[:, :, 0:2]
                nc.scalar.copy(mid, zrow[:, 0:2 * (Hp - 1)].rearrange(
                    "p (h w) -> p h w", w=2))
                h2 = H // 2
                nc.gpsimd.dma_start(out=t[:, 1:h2 + 1, 1:W + 1],
                                    in_=x[img, cc * P:(cc + 1) * P, 0:h2])
                nc.gpsimd.dma_start(out=t[:, h2 + 1:H + 1, 1:W + 1],
                                    in_=x[img, cc * P:(cc + 1) * P, h2:H])
                xt[(img, cc)] = t

        wt = {}

        def prep_w_quarter(oo, cc):
            q2 = QC // 2
            for h in range(2):
                nc.sync.dma_start(
                    out=w_sb[:, oo, cc * QC + h * q2:cc * QC + (h + 1) * q2],
                    in_=w2d[oo * P:(oo + 1) * P,
                            cc * QC + h * q2:cc * QC + (h + 1) * q2])
            nc.vector.tensor_scalar(
                out=sgn_sb[:, oo, cc * QC:(cc + 1) * QC],
                in0=w_sb[:, oo, cc * QC:(cc + 1) * QC],
                scalar1=0.0, scalar2=2.0,
                op0=mybir.AluOpType.is_ge, op1=mybir.AluOpType.mult)
            nc.vector.tensor_scalar_add(
                sgn_sb[:, oo, cc * QC:(cc + 1) * QC],
                sgn_sb[:, oo, cc * QC:(cc + 1) * QC], -1.0)
            for k in range(KK):
                tp = tpool.tile([P, P], dt.bfloat16)
                nc.tensor.transpose(tp, sgn_v[:, oo, cc * P:(cc + 1) * P, k], ident)
                t = wtp.tile([P, P], dt.bfloat16)
                nc.vector.tensor_copy(out=t, in_=tp)
                wt[(oo, cc, k)] = t

        def reduce_scale(oo):
            nc.vector.tensor_reduce(
                out=scale_sb[:, oo:oo + 1], in_=w_sb[:, oo, :],
                axis=mybir.AxisListType.X,
                op=mybir.AluOpType.add, apply_absolute_value=True)
            nc.vector.tensor_scalar_mul(
                scale_sb[:, oo:oo + 1], scale_sb[:, oo:oo + 1], 1.0 / CKK)

        pp = ctx.enter_context(tc.tile_pool(name="psum", bufs=psum_bufs, space="PSUM"))
        op = ctx.enter_context(tc.tile_pool(name="ostage", bufs=ostage_bufs))

        def mm(ps, img, oo, cc, ih, k, n):
            ki, kj = divmod(k, KW)
            rhs = xt[(img, cc)][
                :, ih * hchunk + ki: ih * hchunk + ki + hchunk, kj: kj + W]
            nc.tensor.matmul(ps, lhsT=wt[(oo, cc, k)], rhs=rhs,
                             start=(n == 0), stop=(n == CC * KK - 1))

        def conv_a(img, oo, tiles):
            group = {}
            for ih in tiles:
                ps = pp.tile([P, hchunk * W], dt.float32,
                             name=f"ps_{img}_{oo}_{ih}", tag="ps")
                group[ih] = ps
                for k in range(KK):
                    mm(ps, img, oo, 0, ih, k, n=k)
            return group

        def conv_b(img, oo, group):
            for ih, ps in group.items():
                for k in range(KK):
                    mm(ps, img, oo, 1, ih, k, n=KK + k)
                st = op.tile([P, hchunk * W], dt.float32,
                             name=f"st_{img}_{oo}_{ih}", tag="st")
                nc.scalar.mul(st, ps, scale_sb[:, oo:oo + 1])
                nc.sync.dma_start(
                    out=y[img, oo * P:(oo + 1) * P,
                          ih * hchunk:(ih + 1) * hchunk, :],
                    in_=st)

        def conv(img, oo, skip=0):
            for g0 in range(skip, nch, gsz):
                tiles = list(range(g0, min(g0 + gsz, nch)))
                conv_b(img, oo, conv_a(img, oo, tiles))

        load_x(0)
        prep_w_quarter(0, 0)
        a1 = conv_a(0, 0, list(range(min(gsz, nch))))
        prep_w_quarter(0, 1)
        reduce_scale(0)
        if imgs > 1:
            load_x(1)
        conv_b(0, 0, a1)
        prep_w_quarter(1, 0)
        if nch > gsz:
            a2 = conv_a(0, 0, list(range(gsz, min(2 * gsz, nch))))
            prep_w_quarter(1, 1)
            reduce_scale(1)
            conv_b(0, 0, a2)
            conv(0, 0, skip=2 * gsz)
        else:
            prep_w_quarter(1, 1)
            reduce_scale(1)
        for img in range(2, imgs):
            load_x(img)
        conv(0, 1)
        for img in range(1, imgs):
            conv(img, 0)
            conv(img, 1)
    nc.compile()
    return nc


_NC_CACHE = {}


def _get_nc():
    if "rank1" not in _NC_CACHE:
        _NC_CACHE["rank1"] = _build_rank1_nc(IMGS)
    return _NC_CACHE["rank1"]


def _get_fallback_nc():
    if "conv" not in _NC_CACHE:
        _NC_CACHE["conv"] = _build_conv_nc(IMGS, H, W, hchunk=8, psum_bufs=7,
                                           gsz=4, tp_bufs=1)
    return _NC_CACHE["conv"]


def kernel(**inputs) -> np.ndarray:
    from concourse.bass_utils import run_bass_kernel_spmd

    x = np.ascontiguousarray(np.asarray(inputs["x"], dtype=np.float32))
    weight = np.ascontiguousarray(np.asarray(inputs["weight"], dtype=np.float32))
    assert x.shape == (BATCH, IN_C, H, W), x.shape
    assert weight.shape == (OUT_C * CKK, 1), weight.shape

    # sign(w) == +1 for every weight the module can produce (rand()*1e-3);
    # the rank-1 kernel relies on it, so verify and fall back if violated.
    nc = _get_nc() if (weight >= 0.0).all() else _get_fallback_nc()

    in_maps = [
        {"x": x[c * IMGS:(c + 1) * IMGS], "w": weight}
        for c in range(N_CORES)
    ]
    res = run_bass_kernel_spmd(nc, in_maps, core_ids=list(range(N_CORES)))
    return np.concatenate([res.results[c]["y"] for c in range(N_CORES)], axis=0)


# revision 10
# speedup vs baseline: 2.4636x; 1.0631x over previous
"""Binarized 3x3 conv (GeneralConv2d) on 8 NeuronCores.

y[b,o,h,w] = mean_abs(w[o]) * sum_{c,kh,kw} sign(w[o,c,kh,kw]) * x[b,c,h+kh-1,w+kw-1]

The module initializes w = rand()*0.001, so every weight is non-negative and
sign(w) == +1 identically.  The conv then collapses to a rank-1 form

    y[b,o,:,:] = scale_o * S[b,:,:],   S = 3x3 box filter of sum_c x[b,c]

which is DMA-bound rather than compute-bound.  kernel() verifies the
all-non-negative precondition on the host and falls back to the general
binarized-conv kernel if it ever fails.

Fast path, data-parallel over batch (4 images per core):
 - x is DMA'd f32 with full-width contiguous descriptors into a flat staging
   tile, then converted to bf16 into a zero-padded [128, 58, 58] tile (the
   conversion pass does the padding relayout for free; DMAing the padded
   layout directly would halve DMA bandwidth on 224B descriptors).
 - PE: per 8-row chunk, 6 accumulating bf16 matmuls (2 channel chunks x 3
   vertical taps) with an all-ones stationary operand compute the channel +
   vertical sum, replicated across all 128 partitions, into PSUM [128, 58*8].
 - DVE: copy PSUM->SBUF, then the horizontal 3-tap as two strided adds
   (the zero pad columns make row edges exact).
 - Act: per-partition multiply by scale[oo*128+p] yields the output channel
   chunk directly (the sum is partition-replicated); SP DMAs it out.
"""

import numpy as np

from contextlib import ExitStack

import concourse.bass as bass
import concourse.mybir as mybir
from concourse import bacc
import concourse.tile as tile
from concourse.masks import make_identity

dt = mybir.dt
OUT_C = 256
IN_C = 256
KH = KW = 3
KK = KH * KW           # 9
CKK = IN_C * KK        # 2304
P = 128
CC = IN_C // P         # 2 in-channel chunks
OO = OUT_C // P        # 2 out-channel chunks
QC = CKK // CC         # 1152 columns per (oo,cc) quarter

BATCH, H, W = 32, 56, 56
N_CORES = 8
IMGS = BATCH // N_CORES

HC = 8                 # output rows per PSUM chunk
NCH = H // HC          # 7 chunks per image
Hp = H + 2
Wp = W + 2
FCH = HC * Wp          # 464 PSUM columns per chunk (58-wide rows)


def _build_rank1_nc(imgs: int):
    nc = bacc.Bacc("TRN2", target_bir_lowering=False, debug=False,
                   enable_asserts=False, num_devices=8)
    x = nc.declare_dram_parameter("x", [imgs, IN_C, H, W], dt.float32,
                                  isOutput=False)
    w = nc.declare_dram_parameter("w", [OUT_C * CKK, 1], dt.float32,
                                  isOutput=False)
    y = nc.declare_dram_parameter("y", [imgs, OUT_C, H, W], dt.float32,
                                  isOutput=True)

    x2d = x.rearrange("i c h w -> i c (h w)")
    y2d = y.rearrange("i c h w -> i c (h w)")
    w2d = w.rearrange("(o r) one -> o (r one)", r=CKK)   # [256, 2304]

    with tile.TileContext(nc) as tc, ExitStack() as ctx:
        consts = ctx.enter_context(tc.tile_pool(name="consts", bufs=1))
        ones = consts.tile([P, P], dt.bfloat16)
        nc.vector.memset(ones, 1.0)
        zrow = consts.tile([P, 2 * Wp], dt.bfloat16)
        nc.vector.memset(zrow, 0.0)
        zer = consts.tile([P, FCH], dt.float32)
        nc.vector.memset(zer, 0.0)

        wq = ctx.enter_context(tc.tile_pool(name="wq", bufs=1))
        w_sb = wq.tile([P, OO, CKK], dt.float32)
        scale_sb = wq.tile([P, OO], dt.float32)

        def prep_scale():
            for oo in range(OO):
                nc.sync.dma_start(out=w_sb[:, oo, :],
                                  in_=w2d[oo * P:(oo + 1) * P, :])
            for oo in range(OO):
                nc.vector.tensor_reduce(
                    out=scale_sb[:, oo:oo + 1], in_=w_sb[:, oo, :],
                    axis=mybir.AxisListType.X,
                    op=mybir.AluOpType.add, apply_absolute_value=True)
            nc.vector.tensor_scalar_mul(scale_sb, scale_sb, 1.0 / CKK)

        # f32 staging tiles (full-bandwidth contiguous DMA target).  6 bufs =
        # 3 images in flight, so issue_x(i+2) never waits on convert_x(i+1).
        sxp = ctx.enter_context(tc.tile_pool(name="xstage", bufs=6))
        # bf16 padded tiles; all 8 stay resident so there are no reuse stalls.
        xp = ctx.enter_context(tc.tile_pool(name="xpad", bufs=imgs * CC))
        xt = {}
        stg = {}

        def issue_x(img):
            h2 = H // 2
            for cc in range(CC):
                sx = sxp.tile([P, H * W], dt.float32)
                t = xp.tile([P, Hp, Wp], dt.bfloat16)
                tf = t.rearrange("p h w -> p (h w)")
                # Halo zeroing on Act (idle early): top row, bottom row, and
                # the adjacent (r,57)/(r+1,0) interior column pairs.
                nc.scalar.copy(tf[:, 0:Wp], zrow[:, 0:Wp])
                nc.scalar.copy(tf[:, (Hp - 1) * Wp:Hp * Wp], zrow[:, 0:Wp])
                mid = tf[:, Wp - 1:Wp - 1 + (Hp - 1) * Wp].rearrange(
                    "p (h w) -> p h w", w=Wp)[:, :, 0:2]
                nc.scalar.copy(mid, zrow[:, 0:2 * (Hp - 1)].rearrange(
                    "p (h w) -> p h w", w=2))
                stg[(img, cc)] = (sx, t)
                xt[(img, cc)] = t
            for hh in range(2):
                for cc in range(CC):
                    sx, _ = stg[(img, cc)]
                    nc.gpsimd.dma_start(
                        out=sx[:, hh * h2 * W:(hh + 1) * h2 * W],
                        in_=x2d[img, cc * P:(cc + 1) * P,
                                hh * h2 * W:(hh + 1) * h2 * W])

        def convert_x(img):
            h2 = H // 2
            for hh in range(2):
                for cc in range(CC):
                    sx, t = stg[(img, cc)]
                    nc.gpsimd.tensor_copy(
                        out=t[:, 1 + hh * h2:1 + (hh + 1) * h2, 1:W + 1],
                        in_=sx[:, hh * h2 * W:(hh + 1) * h2 * W].rearrange(
                            "p (h w) -> p h w", w=W))

        pp = ctx.enter_context(tc.tile_pool(name="psum", bufs=8, space="PSUM"))
        vp = ctx.enter_context(tc.tile_pool(name="vtile", bufs=4))
        op = ctx.enter_context(tc.tile_pool(name="stile", bufs=6))

        # Persistent prefix-sum staging slots: the scan writes [1:FCH+1] each
        # chunk; element 0 is the zero base, written exactly once here.
        psc = ctx.enter_context(tc.tile_pool(name="pscan", bufs=1))
        pslots = [psc.tile([P, FCH + 3], dt.float32, name=f"pscan{i}")
                  for i in range(4)]
        for p_ in pslots:
            nc.vector.memset(p_[:, 0:1], 0.0)

        def conv_chunk(img, ih, slot=[0]):
            h0 = ih * HC
            ps = pp.tile([P, FCH], dt.float32, name=f"ps_{img}_{ih}", tag="ps")
            n = 0
            for cc in range(CC):
                t = xt[(img, cc)].rearrange("p h w -> p (h w)")
                for kh in range(KH):
                    base = (h0 + kh) * Wp
                    nc.tensor.matmul(ps, lhsT=ones,
                                     rhs=t[:, base:base + FCH],
                                     start=(n == 0), stop=(n == CC * KH - 1))
                    n += 1
            # ps[p, r*58 + wp] = U[h0+r, wp]: channel+vertical sum (replicated
            # across partitions) over the zero-padded width.  Horizontal
            # 3-tap as differences of the running sum: P[j] = sum_{i<=j} U[i]
            # (stored at p_[j+1]), V[r,w] = P[r*58+w+2] - P[r*58+w-1].
            p_ = pslots[slot[0] % 4]
            slot[0] += 1
            nc.vector.tensor_tensor_scan(
                out=p_[:, 1:FCH + 1], data0=ps, data1=zer, initial=0.0,
                op0=mybir.AluOpType.add, op1=mybir.AluOpType.add)
            hi = p_[:, 3:3 + HC * Wp].rearrange("p (h w) -> p h w", w=Wp)
            lo = p_[:, 0:HC * Wp].rearrange("p (h w) -> p h w", w=Wp)
            v = vp.tile([P, HC, W], dt.float32, name=f"v_{img}_{ih}", tag="v")
            nc.vector.tensor_sub(v, hi[:, :, 0:W], lo[:, :, 0:W])
            vf = v.rearrange("p h w -> p (h w)")
            for oo in range(OO):
                st = op.tile([P, HC * W], dt.float32,
                             name=f"st_{img}_{ih}_{oo}", tag="st")
                nc.scalar.mul(st, vf, scale_sb[:, oo:oo + 1])
                nc.sync.dma_start(
                    out=y2d[img, oo * P:(oo + 1) * P, h0 * W:(h0 + HC) * W],
                    in_=st)

        # Pool program order: DMA issues for image i+1 precede image i's
        # conversions, so loads stream back-to-back and the DMA engines
        # never idle waiting on Pool's in-order sequencer.
        issue_x(0)
        issue_x(1)
        prep_scale()
        convert_x(0)
        issue_x(2)
        convert_x(1)
        for ih in range(NCH):
            conv_chunk(0, ih)
        issue_x(3)
        convert_x(2)
        for ih in range(NCH):
            conv_chunk(1, ih)
        convert_x(3)
        for img in range(2, imgs):
            for ih in range(NCH):
                conv_chunk(img, ih)
    nc.compile()
    return nc


# ---------------------------------------------------------------------------
# General fallback: full binarized conv (sum of 18 shifted GEMMs), used only
# if the weight tensor ever contains a negative value.
# ---------------------------------------------------------------------------


def _build_conv_nc(imgs: int, H: int, W: int, hchunk: int, psum_bufs: int = 7,
                  ostage_bufs: int = 4, gsz: int = 4, tp_bufs: int = 1):
    assert H % hchunk == 0
    nch = H // hchunk
    Hp, Wp = H + 2, W + 2
    nc = bacc.Bacc("TRN2", target_bir_lowering=False, debug=False,
                   enable_asserts=False, num_devices=8)
    x = nc.declare_dram_parameter("x", [imgs, IN_C, H, W], dt.float32, isOutput=False)
    w = nc.declare_dram_parameter("w", [OUT_C * CKK, 1], dt.float32, isOutput=False)
    y = nc.declare_dram_parameter("y", [imgs, OUT_C, H, W], dt.float32, isOutput=True)

    w2d = w.rearrange("(o r) one -> o (r one)", r=CKK)   # [256, 2304]

    with tile.TileContext(nc) as tc, ExitStack() as ctx:
        consts = ctx.enter_context(tc.tile_pool(name="consts", bufs=1))
        ident = consts.tile([P, P], dt.bfloat16)
        make_identity(nc, ident)
        zrow = consts.tile([P, 2 * Wp], dt.bfloat16)
        nc.vector.memset(zrow, 0.0)

        wprep = ctx.enter_context(tc.tile_pool(name="wprep", bufs=1))
        w_sb = wprep.tile([P, OO, CKK], dt.float32)
        sgn_sb = wprep.tile([P, OO, CKK], dt.bfloat16)
        scale_sb = wprep.tile([P, OO], dt.float32)
        sgn_v = sgn_sb.rearrange("p oo (c k) -> p oo c k", k=KK)

        tpool = ctx.enter_context(tc.tile_pool(name="tpsum", bufs=tp_bufs, space="PSUM"))
        wtp = ctx.enter_context(tc.tile_pool(name="wtiles", bufs=OO * CC * KK))
        xp = ctx.enter_context(tc.tile_pool(name="xtiles", bufs=imgs * CC))

        xt = {}

        def load_x(img):
            for cc in range(CC):
                t = xp.tile([P, Hp, Wp], dt.bfloat16)
                tf = t.rearrange("p h w -> p (h w)")
                nc.scalar.copy(tf[:, 0:Wp], zrow[:, 0:Wp])
                nc.scalar.copy(tf[:, (Hp - 1) * Wp:Hp * Wp], zrow[:, 0:Wp])
                mid = tf[:, Wp - 1:Wp - 1 + (Hp - 1) * Wp].rearrange(
                    "p (h w) -> p h w", w=Wp)[:, :, 0:2]
                nc.scalar.copy(mid, zrow[:, 0:2 * (Hp - 1)].rearrange(
                    "p (h w) -> p h w", w=2))
                h2 = H // 2
                nc.gpsimd.dma_start(out=t[:, 1:h2 + 1, 1:W + 1],
                                    in_=x[img, cc * P:(cc + 1) * P, 0:h2])
                nc.gpsimd.dma_start(out=t[:, h2 + 1:H + 1, 1:W + 1],
                                    in_=x[img, cc * P:(cc + 1) * P, h2:H])
                xt[(img, cc)] = t

        wt = {}

        def prep_w_quarter(oo, cc):
            q2 = QC // 2
            for h in range(2):
                nc.sync.dma_start(
                    out=w_sb[:, oo, cc * QC + h * q2:cc * QC + (h + 1) * q2],
                    in_=w2d[oo * P:(oo + 1) * P,
                            cc * QC + h * q2:cc * QC + (h + 1) * q2])
            nc.vector.tensor_scalar(
                out=sgn_sb[:, oo, cc * QC:(cc + 1) * QC],
                in0=w_sb[:, oo, cc * QC:(cc + 1) * QC],
                scalar1=0.0, scalar2=2.0,
                op0=mybir.AluOpType.is_ge, op1=mybir.AluOpType.mult)
            nc.vector.tensor_scalar_add(
                sgn_sb[:, oo, cc * QC:(cc + 1) * QC],
                sgn_sb[:, oo, cc * QC:(cc + 1) * QC], -1.0)
            for k in range(KK):
                tp = tpool.tile([P, P], dt.bfloat16)
                nc.tensor.transpose(tp, sgn_v[:, oo, cc * P:(cc + 1) * P, k], ident)
                t = wtp.tile([P, P], dt.bfloat16)
                nc.vector.tensor_copy(out=t, in_=tp)
                wt[(oo, cc, k)] = t

        def reduce_scale(oo):
            nc.vector.tensor_reduce(
                out=scale_sb[:, oo:oo + 1], in_=w_sb[:, oo, :],
                axis=mybir.AxisListType.X,
                op=mybir.AluOpType.add, apply_absolute_value=True)
            nc.vector.tensor_scalar_mul(
                scale_sb[:, oo:oo + 1], scale_sb[:, oo:oo + 1], 1.0 / CKK)

        pp = ctx.enter_context(tc.tile_pool(name="psum", bufs=psum_bufs, space="PSUM"))
        op = ctx.enter_context(tc.tile_pool(name="ostage", bufs=ostage_bufs))

        def mm(ps, img, oo, cc, ih, k, n):
            ki, kj = divmod(k, KW)
            rhs = xt[(img, cc)][
                :, ih * hchunk + ki: ih * hchunk + ki + hchunk, kj: kj + W]
            nc.tensor.matmul(ps, lhsT=wt[(oo, cc, k)], rhs=rhs,
                             start=(n == 0), stop=(n == CC * KK - 1))

        def conv_a(img, oo, tiles):
            group = {}
            for ih in tiles:
                ps = pp.tile([P, hchunk * W], dt.float32,
                             name=f"ps_{img}_{oo}_{ih}", tag="ps")
                group[ih] = ps
                for k in range(KK):
                    mm(ps, img, oo, 0, ih, k, n=k)
            return group

        def conv_b(img, oo, group):
            for ih, ps in group.items():
                for k in range(KK):
                    mm(ps, img, oo, 1, ih, k, n=KK + k)
                st = op.tile([P, hchunk * W], dt.float32,
                             name=f"st_{img}_{oo}_{ih}", tag="st")
                nc.scalar.mul(st, ps, scale_sb[:, oo:oo + 1])
                nc.sync.dma_start(
                    out=y[img, oo * P:(oo + 1) * P,
                          ih * hchunk:(ih + 1) * hchunk, :],
                    in_=st)

        def conv(img, oo, skip=0):
            for g0 in range(skip, nch, gsz):
                tiles = list(range(g0, min(g0 + gsz, nch)))
                conv_b(img, oo, conv_a(img, oo, tiles))

        load_x(0)
        prep_w_quarter(0, 0)
        a1 = conv_a(0, 0, list(range(min(gsz, nch))))
        prep_w_quarter(0, 1)
        reduce_scale(0)
        if imgs > 1:
            load_x(1)
        conv_b(0, 0, a1)
        prep_w_quarter(1, 0)
        if nch > gsz:
            a2 = conv_a(0, 0, list(range(gsz, min(2 * gsz, nch))))
            prep_w_quarter(1, 1)
            reduce_scale(1)
            conv_b(0, 0, a2)
            conv(0, 0, skip=2 * gsz)
        else:
            prep_w_quarter(1, 1)
            reduce_scale(1)
        for img in range(2, imgs):
            load_x(img)
        conv(0, 1)
        for img in range(1, imgs):
            conv(img, 0)
            conv(img, 1)
    nc.compile()
    return nc


_NC_CACHE = {}


def _get_nc():
    if "rank1" not in _NC_CACHE:
        _NC_CACHE["rank1"] = _build_rank1_nc(IMGS)
    return _NC_CACHE["rank1"]


def _get_fallback_nc():
    if "conv" not in _NC_CACHE:
        _NC_CACHE["conv"] = _build_conv_nc(IMGS, H, W, hchunk=8, psum_bufs=7,
                                           gsz=4, tp_bufs=1)
    return _NC_CACHE["conv"]


def kernel(**inputs) -> np.ndarray:
    from concourse.bass_utils import run_bass_kernel_spmd

    x = np.ascontiguousarray(np.asarray(inputs["x"], dtype=np.float32))
    weight = np.ascontiguousarray(np.asarray(inputs["weight"], dtype=np.float32))
    assert x.shape == (BATCH, IN_C, H, W), x.shape
    assert weight.shape == (OUT_C * CKK, 1), weight.shape

    # sign(w) == +1 for every weight the module can produce (rand()*1e-3);
    # the rank-1 kernel relies on it, so verify and fall back if violated.
    nc = _get_nc() if (weight >= 0.0).all() else _get_fallback_nc()

    in_maps = [
        {"x": x[c * IMGS:(c + 1) * IMGS], "w": weight}
        for c in range(N_CORES)
    ]
    res = run_bass_kernel_spmd(nc, in_maps, core_ids=list(range(N_CORES)))
    return np.concatenate([res.results[c]["y"] for c in range(N_CORES)], axis=0)


# revision 17
# speedup vs baseline: 2.5431x; 1.0323x over previous
"""Binarized 3x3 conv (GeneralConv2d) on 8 NeuronCores.

y[b,o,h,w] = mean_abs(w[o]) * sum_{c,kh,kw} sign(w[o,c,kh,kw]) * x[b,c,h+kh-1,w+kw-1]

The module initializes w = rand()*0.001, so every weight is non-negative and
sign(w) == +1 identically.  The conv then collapses to a rank-1 form

    y[b,o,:,:] = scale_o * S[b,:,:],   S = 3x3 box filter of sum_c x[b,c]

which is DMA-bound rather than compute-bound.  kernel() verifies the
all-non-negative precondition on the host and falls back to the general
binarized-conv kernel if it ever fails.

Fast path, data-parallel over batch (4 images per core):
 - x is DMA'd f32 with full-width contiguous descriptors into a flat staging
   tile, then converted to bf16 into a zero-padded [128, 58, 58] tile (the
   conversion pass does the padding relayout for free; DMAing the padded
   layout directly would halve DMA bandwidth on 224B descriptors).
 - PE: per 8-row chunk, 6 accumulating bf16 matmuls (2 channel chunks x 3
   vertical taps) with an all-ones stationary operand compute the channel +
   vertical sum, replicated across all 128 partitions, into PSUM [128, 58*8].
 - DVE: copy PSUM->SBUF, then the horizontal 3-tap as two strided adds
   (the zero pad columns make row edges exact).
 - Act: per-partition multiply by scale[oo*128+p] yields the output channel
   chunk directly (the sum is partition-replicated); SP DMAs it out.
"""

import numpy as np

from contextlib import ExitStack

import concourse.bass as bass
import concourse.mybir as mybir
from concourse import bacc
import concourse.tile as tile
from concourse.masks import make_identity

dt = mybir.dt
OUT_C = 256
IN_C = 256
KH = KW = 3
KK = KH * KW           # 9
CKK = IN_C * KK        # 2304
P = 128
CC = IN_C // P         # 2 in-channel chunks
OO = OUT_C // P        # 2 out-channel chunks
QC = CKK // CC         # 1152 columns per (oo,cc) quarter

BATCH, H, W = 32, 56, 56
N_CORES = 8
IMGS = BATCH // N_CORES

HC = 8                 # output rows per PSUM chunk
NCH = H // HC          # 7 chunks per image
Hp = H + 2
Wp = W + 2
FCH = HC * Wp          # 464 PSUM columns per chunk (58-wide rows)


def _build_rank1_nc(imgs: int):
    nc = bacc.Bacc("TRN2", target_bir_lowering=False, debug=False,
                   enable_asserts=False, num_devices=8)
    x = nc.declare_dram_parameter("x", [imgs, IN_C, H, W], dt.float32,
                                  isOutput=False)
    w = nc.declare_dram_parameter("w", [OUT_C * CKK, 1], dt.float32,
                                  isOutput=False)
    y = nc.declare_dram_parameter("y", [imgs, OUT_C, H, W], dt.float32,
                                  isOutput=True)

    x2d = x.rearrange("i c h w -> i c (h w)")
    y2d = y.rearrange("i c h w -> i c (h w)")
    y4d = y.rearrange("i (oo c) h w -> i c oo (h w)", oo=OO)
    w2d = w.rearrange("(o r) one -> o (r one)", r=CKK)   # [256, 2304]

    with tile.TileContext(nc) as tc, ExitStack() as ctx:
        consts = ctx.enter_context(tc.tile_pool(name="consts", bufs=1))
        ones = consts.tile([P, P], dt.bfloat16)
        nc.vector.memset(ones, 1.0)
        zrow = consts.tile([P, 2 * Wp], dt.bfloat16)
        nc.vector.memset(zrow, 0.0)
        zer = consts.tile([P, FCH], dt.float32)
        nc.vector.memset(zer, 0.0)

        wq = ctx.enter_context(tc.tile_pool(name="wq", bufs=1))
        w_sb = wq.tile([P, OO, CKK], dt.float32)
        scale_sb = wq.tile([P, OO], dt.float32)

        def prep_scale():
            for oo in range(OO):
                nc.sync.dma_start(out=w_sb[:, oo, :],
                                  in_=w2d[oo * P:(oo + 1) * P, :])
            for oo in range(OO):
                nc.vector.tensor_reduce(
                    out=scale_sb[:, oo:oo + 1], in_=w_sb[:, oo, :],
                    axis=mybir.AxisListType.X,
                    op=mybir.AluOpType.add, apply_absolute_value=True)
            nc.vector.tensor_scalar_mul(scale_sb, scale_sb, 1.0 / CKK)

        # f32 staging tiles (full-bandwidth contiguous DMA target).  6 bufs =
        # 3 images in flight, so issue_x(i+2) never waits on convert_x(i+1).
        sxp = ctx.enter_context(tc.tile_pool(name="xstage", bufs=6))
        # bf16 padded tiles; all 8 stay resident so there are no reuse stalls.
        xp = ctx.enter_context(tc.tile_pool(name="xpad", bufs=imgs * CC))
        xt = {}
        stg = {}

        def issue_x(img):
            h2 = H // 2
            for cc in range(CC):
                sx = sxp.tile([P, H * W], dt.float32)
                t = xp.tile([P, Hp, Wp], dt.bfloat16)
                tf = t.rearrange("p h w -> p (h w)")
                # Halo zeroing on Act (idle early): top row, bottom row, and
                # the adjacent (r,57)/(r+1,0) interior column pairs.
                nc.scalar.copy(tf[:, 0:Wp], zrow[:, 0:Wp])
                nc.scalar.copy(tf[:, (Hp - 1) * Wp:Hp * Wp], zrow[:, 0:Wp])
                mid = tf[:, Wp - 1:Wp - 1 + (Hp - 1) * Wp].rearrange(
                    "p (h w) -> p h w", w=Wp)[:, :, 0:2]
                nc.scalar.copy(mid, zrow[:, 0:2 * (Hp - 1)].rearrange(
                    "p (h w) -> p h w", w=2))
                stg[(img, cc)] = (sx, t)
                xt[(img, cc)] = t
            for hh in range(2):
                for cc in range(CC):
                    sx, _ = stg[(img, cc)]
                    nc.gpsimd.dma_start(
                        out=sx[:, hh * h2 * W:(hh + 1) * h2 * W],
                        in_=x2d[img, cc * P:(cc + 1) * P,
                                hh * h2 * W:(hh + 1) * h2 * W])

        def convert_x(img):
            h2 = H // 2
            for hh in range(2):
                for cc in range(CC):
                    sx, t = stg[(img, cc)]
                    nc.gpsimd.tensor_copy(
                        out=t[:, 1 + hh * h2:1 + (hh + 1) * h2, 1:W + 1],
                        in_=sx[:, hh * h2 * W:(hh + 1) * h2 * W].rearrange(
                            "p (h w) -> p h w", w=W))

        pp = ctx.enter_context(tc.tile_pool(name="psum", bufs=8, space="PSUM"))
        vp = ctx.enter_context(tc.tile_pool(name="vtile", bufs=4))
        op = ctx.enter_context(tc.tile_pool(name="stile", bufs=6))

        # Persistent prefix-sum staging slots: the scan writes [1:FCH+1] each
        # chunk; element 0 is the zero base, written exactly once here.
        psc = ctx.enter_context(tc.tile_pool(name="pscan", bufs=1))
        pslots = [psc.tile([P, FCH + 3], dt.float32, name=f"pscan{i}")
                  for i in range(4)]
        for p_ in pslots:
            nc.vector.memset(p_[:, 0:1], 0.0)

        def conv_chunk(img, ih, slot=[0]):
            h0 = ih * HC
            ps = pp.tile([P, FCH], dt.float32, name=f"ps_{img}_{ih}", tag="ps")
            n = 0
            for cc in range(CC):
                t = xt[(img, cc)].rearrange("p h w -> p (h w)")
                for kh in range(KH):
                    base = (h0 + kh) * Wp
                    nc.tensor.matmul(ps, lhsT=ones,
                                     rhs=t[:, base:base + FCH],
                                     start=(n == 0), stop=(n == CC * KH - 1))
                    n += 1
            # ps[p, r*58 + wp] = U[h0+r, wp]: channel+vertical sum (replicated
            # across partitions) over the zero-padded width.  Horizontal
            # 3-tap as differences of the running sum: P[j] = sum_{i<=j} U[i]
            # (stored at p_[j+1]), V[r,w] = P[r*58+w+2] - P[r*58+w-1].
            p_ = pslots[slot[0] % 4]
            slot[0] += 1
            nc.vector.tensor_tensor_scan(
                out=p_[:, 1:FCH + 1], data0=ps, data1=zer, initial=0.0,
                op0=mybir.AluOpType.add, op1=mybir.AluOpType.add)
            hi = p_[:, 3:3 + HC * Wp].rearrange("p (h w) -> p h w", w=Wp)
            lo = p_[:, 0:HC * Wp].rearrange("p (h w) -> p h w", w=Wp)
            v = vp.tile([P, HC, W], dt.float32, name=f"v_{img}_{ih}", tag="v")
            nc.vector.tensor_sub(v, hi[:, :, 0:W], lo[:, :, 0:W])
            vf = v.rearrange("p h w -> p (h w)")
            # Both out-channel chunks staged side by side and shipped in ONE
            # 3D-AP DMA (partition p, oo, hw) — halves store issues.
            st = op.tile([P, OO, HC * W], dt.float32,
                         name=f"st_{img}_{ih}", tag="st")
            for oo in range(OO):
                nc.scalar.mul(st[:, oo, :], vf, scale_sb[:, oo:oo + 1])
            nc.sync.dma_start(
                out=y4d[img, :, :, h0 * W:(h0 + HC) * W], in_=st)

        # Pool program order: DMA issues for image i+1 precede image i's
        # conversions, so loads stream back-to-back and the DMA engines
        # never idle waiting on Pool's in-order sequencer.
        issue_x(0)
        issue_x(1)
        prep_scale()
        convert_x(0)
        issue_x(2)
        convert_x(1)
        for ih in range(NCH):
            conv_chunk(0, ih)
        issue_x(3)
        convert_x(2)
        for ih in range(NCH):
            conv_chunk(1, ih)
        convert_x(3)
        for img in range(2, imgs):
            for ih in range(NCH):
                conv_chunk(img, ih)
    nc.compile()
    return nc


# ---------------------------------------------------------------------------
# General fallback: full binarized conv (sum of 18 shifted GEMMs), used only
# if the weight tensor ever contains a negative value.
# ---------------------------------------------------------------------------


def _build_conv_nc(imgs: int, H: int, W: int, hchunk: int, psum_bufs: int = 7,
                  ostage_bufs: int = 4, gsz: int = 4, tp_bufs: int = 1):
    assert H % hchunk == 0
    nch = H // hchunk
    Hp, Wp = H + 2, W + 2
    nc = bacc.Bacc("TRN2", target_bir_lowering=False, debug=False,
                   enable_asserts=False, num_devices=8)
    x = nc.declare_dram_parameter("x", [imgs, IN_C, H, W], dt.float32, isOutput=False)
    w = nc.declare_dram_parameter("w", [OUT_C * CKK, 1], dt.float32, isOutput=False)
    y = nc.declare_dram_parameter("y", [imgs, OUT_C, H, W], dt.float32, isOutput=True)

    w2d = w.rearrange("(o r) one -> o (r one)", r=CKK)   # [256, 2304]

    with tile.TileContext(nc) as tc, ExitStack() as ctx:
        consts = ctx.enter_context(tc.tile_pool(name="consts", bufs=1))
        ident = consts.tile([P, P], dt.bfloat16)
        make_identity(nc, ident)
        zrow = consts.tile([P, 2 * Wp], dt.bfloat16)
        nc.vector.memset(zrow, 0.0)

        wprep = ctx.enter_context(tc.tile_pool(name="wprep", bufs=1))
        w_sb = wprep.tile([P, OO, CKK], dt.float32)
        sgn_sb = wprep.tile([P, OO, CKK], dt.bfloat16)
        scale_sb = wprep.tile([P, OO], dt.float32)
        sgn_v = sgn_sb.rearrange("p oo (c k) -> p oo c k", k=KK)

        tpool = ctx.enter_context(tc.tile_pool(name="tpsum", bufs=tp_bufs, space="PSUM"))
        wtp = ctx.enter_context(tc.tile_pool(name="wtiles", bufs=OO * CC * KK))
        xp = ctx.enter_context(tc.tile_pool(name="xtiles", bufs=imgs * CC))

        xt = {}

        def load_x(img):
            for cc in range(CC):
                t = xp.tile([P, Hp, Wp], dt.bfloat16)
                tf = t.rearrange("p h w -> p (h w)")
                nc.scalar.copy(tf[:, 0:Wp], zrow[:, 0:Wp])
                nc.scalar.copy(tf[:, (Hp - 1) * Wp:Hp * Wp], zrow[:, 0:Wp])
                mid = tf[:, Wp - 1:Wp - 1 + (Hp - 1) * Wp].rearrange(
                    "p (h w) -> p h w", w=Wp)[:, :, 0:2]
                nc.scalar.copy(mid, zrow[:, 0:2 * (Hp - 1)].rearrange(
                    "p (h w) -> p h w", w=2))
                h2 = H // 2
                nc.gpsimd.dma_start(out=t[:, 1:h2 + 1, 1:W + 1],
                                    in_=x[img, cc * P:(cc + 1) * P, 0:h2])
                nc.gpsimd.dma_start(out=t[:, h2 + 1:H + 1, 1:W + 1],
                                    in_=x[img, cc * P:(cc + 1) * P, h2:H])
                xt[(img, cc)] = t

        wt = {}

        def prep_w_quarter(oo, cc):
            q2 = QC // 2
            for h in range(2):
                nc.sync.dma_start(
                    out=w_sb[:, oo, cc * QC + h * q2:cc * QC + (h + 1) * q2],
                    in_=w2d[oo * P:(oo + 1) * P,
                            cc * QC + h * q2:cc * QC + (h + 1) * q2])
            nc.vector.tensor_scalar(
                out=sgn_sb[:, oo, cc * QC:(cc + 1) * QC],
                in0=w_sb[:, oo, cc * QC:(cc + 1) * QC],
                scalar1=0.0, scalar2=2.0,
                op0=mybir.AluOpType.is_ge, op1=mybir.AluOpType.mult)
            nc.vector.tensor_scalar_add(
                sgn_sb[:, oo, cc * QC:(cc + 1) * QC],
                sgn_sb[:, oo, cc * QC:(cc + 1) * QC], -1.0)
            for k in range(KK):
                tp = tpool.tile([P, P], dt.bfloat16)
                nc.tensor.transpose(tp, sgn_v[:, oo, cc * P:(cc + 1) * P, k], ident)
                t = wtp.tile([P, P], dt.bfloat16)
                nc.vector.tensor_copy(out=t, in_=tp)
                wt[(oo, cc, k)] = t

        def reduce_scale(oo):
            nc.vector.tensor_reduce(
                out=scale_sb[:, oo:oo + 1], in_=w_sb[:, oo, :],
                axis=mybir.AxisListType.X,
                op=mybir.AluOpType.add, apply_absolute_value=True)
            nc.vector.tensor_scalar_mul(
                scale_sb[:, oo:oo + 1], scale_sb[:, oo:oo + 1], 1.0 / CKK)

        pp = ctx.enter_context(tc.tile_pool(name="psum", bufs=psum_bufs, space="PSUM"))
        op = ctx.enter_context(tc.tile_pool(name="ostage", bufs=ostage_bufs))

        def mm(ps, img, oo, cc, ih, k, n):
            ki, kj = divmod(k, KW)
            rhs = xt[(img, cc)][
                :, ih * hchunk + ki: ih * hchunk + ki + hchunk, kj: kj + W]
            nc.tensor.matmul(ps, lhsT=wt[(oo, cc, k)], rhs=rhs,
                             start=(n == 0), stop=(n == CC * KK - 1))

        def conv_a(img, oo, tiles):
            group = {}
            for ih in tiles:
                ps = pp.tile([P, hchunk * W], dt.float32,
                             name=f"ps_{img}_{oo}_{ih}", tag="ps")
                group[ih] = ps
                for k in range(KK):
                    mm(ps, img, oo, 0, ih, k, n=k)
            return group

        def conv_b(img, oo, group):
            for ih, ps in group.items():
                for k in range(KK):
                    mm(ps, img, oo, 1, ih, k, n=KK + k)
                st = op.tile([P, hchunk * W], dt.float32,
                             name=f"st_{img}_{oo}_{ih}", tag="st")
                nc.scalar.mul(st, ps, scale_sb[:, oo:oo + 1])
                nc.sync.dma_start(
                    out=y[img, oo * P:(oo + 1) * P,
                          ih * hchunk:(ih + 1) * hchunk, :],
                    in_=st)

        def conv(img, oo, skip=0):
            for g0 in range(skip, nch, gsz):
                tiles = list(range(g0, min(g0 + gsz, nch)))
                conv_b(img, oo, conv_a(img, oo, tiles))

        load_x(0)
        prep_w_quarter(0, 0)
        a1 = conv_a(0, 0, list(range(min(gsz, nch))))
        prep_w_quarter(0, 1)
        reduce_scale(0)
        if imgs > 1:
            load_x(1)
        conv_b(0, 0, a1)
        prep_w_quarter(1, 0)
        if nch > gsz:
            a2 = conv_a(0, 0, list(range(gsz, min(2 * gsz, nch))))
            prep_w_quarter(1, 1)
            reduce_scale(1)
            conv_b(0, 0, a2)
            conv(0, 0, skip=2 * gsz)
        else:
            prep_w_quarter(1, 1)
            reduce_scale(1)
        for img in range(2, imgs):
            load_x(img)
        conv(0, 1)
        for img in range(1, imgs):
            conv(img, 0)
            conv(img, 1)
    nc.compile()
    return nc


_NC_CACHE = {}


def _get_nc():
    if "rank1" not in _NC_CACHE:
        _NC_CACHE["rank1"] = _build_rank1_nc(IMGS)
    return _NC_CACHE["rank1"]


def _get_fallback_nc():
    if "conv" not in _NC_CACHE:
        _NC_CACHE["conv"] = _build_conv_nc(IMGS, H, W, hchunk=8, psum_bufs=7,
                                           gsz=4, tp_bufs=1)
    return _NC_CACHE["conv"]


def kernel(**inputs) -> np.ndarray:
    from concourse.bass_utils import run_bass_kernel_spmd

    x = np.ascontiguousarray(np.asarray(inputs["x"], dtype=np.float32))
    weight = np.ascontiguousarray(np.asarray(inputs["weight"], dtype=np.float32))
    assert x.shape == (BATCH, IN_C, H, W), x.shape
    assert weight.shape == (OUT_C * CKK, 1), weight.shape

    # sign(w) == +1 for every weight the module can produce (rand()*1e-3);
    # the rank-1 kernel relies on it, so verify and fall back if violated.
    fast = bool((weight >= 0.0).all())
    nc = _get_nc() if fast else _get_fallback_nc()

    in_maps = [
        {"x": x[c * IMGS:(c + 1) * IMGS], "w": weight}
        for c in range(N_CORES)
    ]
    res = run_bass_kernel_spmd(nc, in_maps, core_ids=list(range(N_CORES)))
    return np.concatenate([res.results[c]["y"] for c in range(N_CORES)], axis=0)


# revision 34
# speedup vs baseline: 4.2857x; 1.6852x over previous
"""Binarized 3x3 conv (GeneralConv2d) on 8 NeuronCores.

y[b,o,h,w] = mean_abs(w[o]) * sum_{c,kh,kw} sign(w[o,c,kh,kw]) * x[b,c,h+kh-1,w+kw-1]

The module initializes w = rand()*0.001, so every weight is non-negative and
sign(w) == +1 identically.  The conv then collapses to a rank-1 form

    y[b,o,:,:] = scale_o * S[b,:,:],   S = 3x3 box filter of sum_c x[b,c]

which is DMA-bound rather than compute-bound.  kernel() verifies the
all-non-negative precondition on the host and falls back to the general
binarized-conv kernel if it ever fails.

Fast path, data-parallel over batch (4 images per core):
 - x is DMA'd f32 with full-width contiguous descriptors into a flat staging
   tile, then converted to bf16 into a zero-padded [128, 58, 58] tile (the
   conversion pass does the padding relayout for free; DMAing the padded
   layout directly would halve DMA bandwidth on 224B descriptors).
 - PE: per 8-row chunk, 6 accumulating bf16 matmuls (2 channel chunks x 3
   vertical taps) with an all-ones stationary operand compute the channel +
   vertical sum, replicated across all 128 partitions, into PSUM [128, 58*8].
 - DVE: copy PSUM->SBUF, then the horizontal 3-tap as two strided adds
   (the zero pad columns make row edges exact).
 - Act: per-partition multiply by scale[oo*128+p] yields the output channel
   chunk directly (the sum is partition-replicated); SP DMAs it out.
"""

import numpy as np

from contextlib import ExitStack

import concourse.bass as bass
import concourse.mybir as mybir
from concourse import bacc
import concourse.tile as tile
from concourse.masks import make_identity

dt = mybir.dt
OUT_C = 256
IN_C = 256
KH = KW = 3
KK = KH * KW           # 9
CKK = IN_C * KK        # 2304
P = 128
CC = IN_C // P         # 2 in-channel chunks
OO = OUT_C // P        # 2 out-channel chunks
QC = CKK // CC         # 1152 columns per (oo,cc) quarter

BATCH, H, W = 32, 56, 56
N_CORES = 8
IMGS = BATCH // N_CORES

HC = 8                 # output rows per PSUM chunk
NCH = H // HC          # 7 chunks per image
Hp = H + 2
Wp = W + 2
FLAT = Hp * W + 2      # flat x tile: 58 rows of 56, +1 slack element each end
FCH = HC * W + 2       # 450 PSUM columns per chunk (1-elem halo each side)


def _build_rank1_nc(imgs: int):
    # All HBM I/O in bf16: the host ships x/w pre-rounded to bf16 (the exact
    # rounding the f32 kernel applied on-device anyway) and upcasts y back to
    # f32.  Halves DMA traffic; output rounding adds ~2^-9 relative error,
    # well inside the 2e-2 gate.
    nc = bacc.Bacc("TRN2", target_bir_lowering=False, debug=False,
                   enable_asserts=False, num_devices=8)
    x = nc.declare_dram_parameter("x", [imgs, IN_C, H, W], dt.bfloat16,
                                  isOutput=False)
    w = nc.declare_dram_parameter("w", [OUT_C * CKK, 1], dt.bfloat16,
                                  isOutput=False)
    y = nc.declare_dram_parameter("y", [imgs, OUT_C, H, W], dt.bfloat16,
                                  isOutput=True)

    x2d = x.rearrange("i c h w -> i c (h w)")
    y4d = y.rearrange("i (oo c) h w -> i c oo (h w)", oo=OO)
    w2d = w.rearrange("(o r) one -> o (r one)", r=CKK)   # [256, 2304]

    with tile.TileContext(nc) as tc, ExitStack() as ctx:
        consts = ctx.enter_context(tc.tile_pool(name="consts", bufs=1))
        ones = consts.tile([P, P], dt.bfloat16)
        nc.vector.memset(ones, 1.0)
        zer = consts.tile([P, FCH], dt.float32)
        nc.vector.memset(zer, 0.0)

        # PE p-state warmup: ~85 back-to-back dummy matmuls keep the tensor
        # engine continuously busy from t~1us until the first x tile lands,
        # so the real matmuls start at the full 2.4 GHz clock instead of
        # ramping from cold.
        wp_ = ctx.enter_context(tc.tile_pool(name="warm", bufs=1, space="PSUM"))
        warm_ps = wp_.tile([P, P], dt.float32)
        for _ in range(85):
            nc.tensor.matmul(warm_ps, lhsT=ones, rhs=ones, start=True,
                             stop=True)

        wq = ctx.enter_context(tc.tile_pool(name="wq", bufs=1))
        w_sb = wq.tile([P, OO, CKK], dt.bfloat16)
        w_scr = wq.tile([P, CKK], dt.bfloat16)
        acc_sb = wq.tile([P, OO], dt.float32)
        scale_sb = wq.tile([P, OO], dt.float32)

        def prep_scale():
            # w DMAs issued from Pool AFTER image 0's loads, so x leads the
            # DMA queue.  The row sums run on Act (idle until the first
            # scale-muls) via the activation accumulator; w >= 0 on this path
            # so sum == sum(|.|).
            for oo in range(OO):
                nc.gpsimd.dma_start(out=w_sb[:, oo, :],
                                    in_=w2d[oo * P:(oo + 1) * P, :])
            for oo in range(OO):
                nc.scalar.activation(
                    out=w_scr, in_=w_sb[:, oo, :],
                    func=mybir.ActivationFunctionType.Copy,
                    accum_out=acc_sb[:, oo:oo + 1])
            nc.scalar.mul(scale_sb, acc_sb, 1.0 / CKK)

        # Flat padded x tiles [128, 1 + 58*56 + 1] bf16: one zero row above and
        # below the image plus one slack element each end.  The bf16 DMA lands
        # the whole image contiguously (6272B descriptors, full bandwidth) —
        # no staging or relayout pass.  All 8 tiles stay resident.
        xp = ctx.enter_context(tc.tile_pool(name="xflat", bufs=imgs * CC))
        xt = {}

        def issue_x(img):
            for cc in range(CC):
                t = xp.tile([P, FLAT], dt.bfloat16)
                # Vertical halo zeroing on Pool: slack + top pad row, bottom
                # pad row + slack.
                nc.gpsimd.memset(t[:, 0:W + 1], 0.0)
                nc.gpsimd.memset(t[:, FLAT - (W + 1):FLAT], 0.0)
                # Image 0 loads go through SP's HWDGE (fastest issue path) so
                # the pipeline head starts ~0.7us earlier; the rest go through
                # Pool so they never queue behind store issues.
                eng = nc.sync if img == 0 else nc.gpsimd
                eng.dma_start(out=t[:, W + 1:W + 1 + H * W],
                              in_=x2d[img, cc * P:(cc + 1) * P, :])
                xt[(img, cc)] = t

        pp = ctx.enter_context(tc.tile_pool(name="psum", bufs=7, space="PSUM"))
        vp = ctx.enter_context(tc.tile_pool(name="vtile", bufs=4))
        op = ctx.enter_context(tc.tile_pool(name="stile", bufs=6))

        # Persistent prefix-sum staging slots, one pair of scan lanes each:
        # the scans write [g, 1:FCH+1]; element [g, 0] is the zero base,
        # written exactly once here.
        psc = ctx.enter_context(tc.tile_pool(name="pscan", bufs=1))
        pslots = [psc.tile([P, 2, FCH + 1], dt.float32, name=f"pscan{i}")
                  for i in range(4)]
        for p_ in pslots:
            nc.vector.memset(p_[:, :, 0:1], 0.0)

        def conv_group(img, ihs, slot=[0]):
            # Process up to 2 chunks as one group: per chunk, 6 accumulating
            # matmuls into an own PSUM bank and a DVE prefix-scan; then ONE
            # fused Pool subtract, ONE Act mul per out-chunk, ONE store DMA.
            ng = len(ihs)
            p_ = pslots[slot[0] % 4]
            slot[0] += 1
            for g, ih in enumerate(ihs):
                h0 = ih * HC
                ps = pp.tile([P, FCH], dt.float32,
                             name=f"ps_{img}_{ih}", tag="ps")
                n = 0
                for cc in range(CC):
                    t = xt[(img, cc)]
                    for kh in range(KH):
                        base = (h0 + kh) * W
                        nc.tensor.matmul(ps, lhsT=ones,
                                         rhs=t[:, base:base + FCH],
                                         start=(n == 0),
                                         stop=(n == CC * KH - 1))
                        n += 1
                # ps[p, j] = U[h0*56 - 1 + j]: channel+vertical sum,
                # replicated across partitions, on the flat 56-wide layout.
                nc.vector.tensor_tensor_scan(
                    out=p_[:, g, 1:FCH + 1], data0=ps, data1=zer, initial=0.0,
                    op0=mybir.AluOpType.add, op1=mybir.AluOpType.add)
            # Horizontal 3-tap as differences of the running sum P (stored at
            # p_[j+1]): flat shifts wrap across rows; the two edge columns of
            # each row are recomputed from the correct prefix differences.
            GW = HC * W
            pg = p_[:, 0:ng, :]
            v = vp.tile([P, ng, GW], dt.float32, name=f"v_{img}_{ihs[0]}",
                        tag="v")
            nc.vector.tensor_sub(v, pg[:, :, 3:3 + GW], pg[:, :, 0:GW])
            v4 = v.rearrange("p g (h w) -> p g h w", w=W)
            e3 = pg[:, :, 3:3 + GW].rearrange("p g (h w) -> p g h w", w=W)
            e2 = pg[:, :, 2:2 + GW].rearrange("p g (h w) -> p g h w", w=W)
            e1 = pg[:, :, 1:1 + GW].rearrange("p g (h w) -> p g h w", w=W)
            e0 = pg[:, :, 0:GW].rearrange("p g (h w) -> p g h w", w=W)
            # w=0:  V = P[k+2]-P[k]   = p_[k+3]-p_[k+1]
            nc.gpsimd.tensor_sub(v4[:, :, :, 0:1], e3[:, :, :, 0:1],
                                 e1[:, :, :, 0:1])
            # w=55: V = P[k+1]-P[k-1] = p_[k+2]-p_[k]
            nc.gpsimd.tensor_sub(v4[:, :, :, W - 1:W], e2[:, :, :, W - 1:W],
                                 e0[:, :, :, W - 1:W])
            # Both out-channel chunks staged side by side and shipped in ONE
            # 3D-AP DMA (partition p, oo, hw).  bf16 out: the scale-mul does
            # the downconversion for free.
            vf = v.rearrange("p g hw -> p (g hw)")
            st = op.tile([P, OO, ng * GW], dt.bfloat16,
                         name=f"st_{img}_{ihs[0]}", tag="st")
            for oo in range(OO):
                nc.scalar.mul(st[:, oo, :], vf, scale_sb[:, oo:oo + 1])
            h0 = ihs[0] * HC
            nc.sync.dma_start(
                out=y4d[img, :, :, h0 * W:(h0 + ng * HC) * W], in_=st)

        issue_x(0)
        prep_scale()
        issue_x(1)
        issue_x(2)
        issue_x(3)
        for img in range(imgs):
            for ih0 in range(0, NCH, 2):
                conv_group(img, list(range(ih0, min(ih0 + 2, NCH))))
    nc.compile()
    return nc


# ---------------------------------------------------------------------------
# General fallback: full binarized conv (sum of 18 shifted GEMMs), used only
# if the weight tensor ever contains a negative value.
# ---------------------------------------------------------------------------


def _build_conv_nc(imgs: int, H: int, W: int, hchunk: int, psum_bufs: int = 7,
                  ostage_bufs: int = 4, gsz: int = 4, tp_bufs: int = 1):
    assert H % hchunk == 0
    nch = H // hchunk
    Hp, Wp = H + 2, W + 2
    nc = bacc.Bacc("TRN2", target_bir_lowering=False, debug=False,
                   enable_asserts=False, num_devices=8)
    x = nc.declare_dram_parameter("x", [imgs, IN_C, H, W], dt.float32, isOutput=False)
    w = nc.declare_dram_parameter("w", [OUT_C * CKK, 1], dt.float32, isOutput=False)
    y = nc.declare_dram_parameter("y", [imgs, OUT_C, H, W], dt.float32, isOutput=True)

    w2d = w.rearrange("(o r) one -> o (r one)", r=CKK)   # [256, 2304]

    with tile.TileContext(nc) as tc, ExitStack() as ctx:
        consts = ctx.enter_context(tc.tile_pool(name="consts", bufs=1))
        ident = consts.tile([P, P], dt.bfloat16)
        make_identity(nc, ident)
        zrow = consts.tile([P, 2 * Wp], dt.bfloat16)
        nc.vector.memset(zrow, 0.0)

        wprep = ctx.enter_context(tc.tile_pool(name="wprep", bufs=1))
        w_sb = wprep.tile([P, OO, CKK], dt.float32)
        sgn_sb = wprep.tile([P, OO, CKK], dt.bfloat16)
        scale_sb = wprep.tile([P, OO], dt.float32)
        sgn_v = sgn_sb.rearrange("p oo (c k) -> p oo c k", k=KK)

        tpool = ctx.enter_context(tc.tile_pool(name="tpsum", bufs=tp_bufs, space="PSUM"))
        wtp = ctx.enter_context(tc.tile_pool(name="wtiles", bufs=OO * CC * KK))
        xp = ctx.enter_context(tc.tile_pool(name="xtiles", bufs=imgs * CC))

        xt = {}

        def load_x(img):
            for cc in range(CC):
                t = xp.tile([P, Hp, Wp], dt.bfloat16)
                tf = t.rearrange("p h w -> p (h w)")
                nc.scalar.copy(tf[:, 0:Wp], zrow[:, 0:Wp])
                nc.scalar.copy(tf[:, (Hp - 1) * Wp:Hp * Wp], zrow[:, 0:Wp])
                mid = tf[:, Wp - 1:Wp - 1 + (Hp - 1) * Wp].rearrange(
                    "p (h w) -> p h w", w=Wp)[:, :, 0:2]
                nc.scalar.copy(mid, zrow[:, 0:2 * (Hp - 1)].rearrange(
                    "p (h w) -> p h w", w=2))
                h2 = H // 2
                nc.gpsimd.dma_start(out=t[:, 1:h2 + 1, 1:W + 1],
                                    in_=x[img, cc * P:(cc + 1) * P, 0:h2])
                nc.gpsimd.dma_start(out=t[:, h2 + 1:H + 1, 1:W + 1],
                                    in_=x[img, cc * P:(cc + 1) * P, h2:H])
                xt[(img, cc)] = t

        wt = {}

        def prep_w_quarter(oo, cc):
            q2 = QC // 2
            for h in range(2):
                nc.sync.dma_start(
                    out=w_sb[:, oo, cc * QC + h * q2:cc * QC + (h + 1) * q2],
                    in_=w2d[oo * P:(oo + 1) * P,
                            cc * QC + h * q2:cc * QC + (h + 1) * q2])
            nc.vector.tensor_scalar(
                out=sgn_sb[:, oo, cc * QC:(cc + 1) * QC],
                in0=w_sb[:, oo, cc * QC:(cc + 1) * QC],
                scalar1=0.0, scalar2=2.0,
                op0=mybir.AluOpType.is_ge, op1=mybir.AluOpType.mult)
            nc.vector.tensor_scalar_add(
                sgn_sb[:, oo, cc * QC:(cc + 1) * QC],
                sgn_sb[:, oo, cc * QC:(cc + 1) * QC], -1.0)
            for k in range(KK):
                tp = tpool.tile([P, P], dt.bfloat16)
                nc.tensor.transpose(tp, sgn_v[:, oo, cc * P:(cc + 1) * P, k], ident)
                t = wtp.tile([P, P], dt.bfloat16)
                nc.vector.tensor_copy(out=t, in_=tp)
                wt[(oo, cc, k)] = t

        def reduce_scale(oo):
            nc.vector.tensor_reduce(
                out=scale_sb[:, oo:oo + 1], in_=w_sb[:, oo, :],
                axis=mybir.AxisListType.X,
                op=mybir.AluOpType.add, apply_absolute_value=True)
            nc.vector.tensor_scalar_mul(
                scale_sb[:, oo:oo + 1], scale_sb[:, oo:oo + 1], 1.0 / CKK)

        pp = ctx.enter_context(tc.tile_pool(name="psum", bufs=psum_bufs, space="PSUM"))
        op = ctx.enter_context(tc.tile_pool(name="ostage", bufs=ostage_bufs))

        def mm(ps, img, oo, cc, ih, k, n):
            ki, kj = divmod(k, KW)
            rhs = xt[(img, cc)][
                :, ih * hchunk + ki: ih * hchunk + ki + hchunk, kj: kj + W]
            nc.tensor.matmul(ps, lhsT=wt[(oo, cc, k)], rhs=rhs,
                             start=(n == 0), stop=(n == CC * KK - 1))

        def conv_a(img, oo, tiles):
            group = {}
            for ih in tiles:
                ps = pp.tile([P, hchunk * W], dt.float32,
                             name=f"ps_{img}_{oo}_{ih}", tag="ps")
                group[ih] = ps
                for k in range(KK):
                    mm(ps, img, oo, 0, ih, k, n=k)
            return group

        def conv_b(img, oo, group):
            for ih, ps in group.items():
                for k in range(KK):
                    mm(ps, img, oo, 1, ih, k, n=KK + k)
                st = op.tile([P, hchunk * W], dt.float32,
                             name=f"st_{img}_{oo}_{ih}", tag="st")
                nc.scalar.mul(st, ps, scale_sb[:, oo:oo + 1])
                nc.sync.dma_start(
                    out=y[img, oo * P:(oo + 1) * P,
                          ih * hchunk:(ih + 1) * hchunk, :],
                    in_=st)

        def conv(img, oo, skip=0):
            for g0 in range(skip, nch, gsz):
                tiles = list(range(g0, min(g0 + gsz, nch)))
                conv_b(img, oo, conv_a(img, oo, tiles))

        load_x(0)
        prep_w_quarter(0, 0)
        a1 = conv_a(0, 0, list(range(min(gsz, nch))))
        prep_w_quarter(0, 1)
        reduce_scale(0)
        if imgs > 1:
            load_x(1)
        conv_b(0, 0, a1)
        prep_w_quarter(1, 0)
        if nch > gsz:
            a2 = conv_a(0, 0, list(range(gsz, min(2 * gsz, nch))))
            prep_w_quarter(1, 1)
            reduce_scale(1)
            conv_b(0, 0, a2)
            conv(0, 0, skip=2 * gsz)
        else:
            prep_w_quarter(1, 1)
            reduce_scale(1)
        for img in range(2, imgs):
            load_x(img)
        conv(0, 1)
        for img in range(1, imgs):
            conv(img, 0)
            conv(img, 1)
    nc.compile()
    return nc


_NC_CACHE = {}


def _get_nc():
    if "rank1" not in _NC_CACHE:
        _NC_CACHE["rank1"] = _build_rank1_nc(IMGS)
    return _NC_CACHE["rank1"]


def _get_fallback_nc():
    if "conv" not in _NC_CACHE:
        _NC_CACHE["conv"] = _build_conv_nc(IMGS, H, W, hchunk=8, psum_bufs=7,
                                           gsz=4, tp_bufs=1)
    return _NC_CACHE["conv"]


def kernel(**inputs) -> np.ndarray:
    from concourse.bass_utils import run_bass_kernel_spmd

    x = np.ascontiguousarray(np.asarray(inputs["x"], dtype=np.float32))
    weight = np.ascontiguousarray(np.asarray(inputs["weight"], dtype=np.float32))
    assert x.shape == (BATCH, IN_C, H, W), x.shape
    assert weight.shape == (OUT_C * CKK, 1), weight.shape

    # sign(w) == +1 for every weight the module can produce (rand()*1e-3);
    # the rank-1 kernel relies on it, so verify and fall back if violated.
    fast = bool((weight >= 0.0).all())
    nc = _get_nc() if fast else _get_fallback_nc()

    if fast:
        bf16 = dt.np(dt.bfloat16)
        xb = x.astype(bf16)
        wb = weight.astype(bf16)
        in_maps = [
            {"x": xb[c * IMGS:(c + 1) * IMGS], "w": wb}
            for c in range(N_CORES)
        ]
    else:
        in_maps = [
            {"x": x[c * IMGS:(c + 1) * IMGS], "w": weight}
            for c in range(N_CORES)
        ]
    res = run_bass_kernel_spmd(nc, in_maps, core_ids=list(range(N_CORES)))
    out = np.concatenate([res.results[c]["y"] for c in range(N_CORES)], axis=0)
    return np.ascontiguousarray(out.astype(np.float32))
